# revision 3
# baseline (speedup 1.0000x reference)
"""DeltaQuantLinear kernel for 8 Trainium2 NeuronCores.

Computes out = x @ (base_weight + (q_delta - zp[:,None]) * scale[:,None]).T + bias
with x [8, 4096] fp32, base_weight/q_delta [11008, 4096], per-channel
scales/zero_points/bias [11008].

Strategy (column-parallel over out_features, per the sharding hint):
  The dequant folds into pure GEMM algebra:
      out[t,o] = sum_i x[t,i]*base[o,i] + sum_i x[t,i]*(q[o,i]*scale[o])
               + (bias[o] - scale[o]*zp[o]*S[t]),   S[t] = sum_i x[t,i]
  so the device kernel is a single memory-bound fp32 matmul streaming the
  (host-side pre-transposed, pre-scaled, concatenated) weights once through
  the PE with the tiny x operand stationary. Per core: [8192, 1376] weights,
  3 PSUM accumulators, one elementwise bias add, out.
"""

import numpy as np

from concourse import bacc, bass, mybir, tile
from concourse import bass_utils

IN_F = 4096
OUT_F = 11008
TOKENS = 8
NCORES = 8
SHARD = OUT_F // NCORES          # 1376
K2 = 2 * IN_F                    # 8192 contract dim (base ++ scaled q)
NCHUNK = K2 // 128               # 64 chunks of 128
# o-tile split of the 1376-wide shard into <=512 psum banks
O_SPLITS = [(0, 512), (512, 512), (1024, 352)]

F32 = mybir.dt.float32
F32R = mybir.dt.float32r

_CACHE = {}

# test.py reads this after calling kernel() to get profile info
LAST_RESULTS = None
TRACE = False


def _build_nc():
    nc = bacc.Bacc(
        "TRN2",
        target_bir_lowering=False,
        debug=False,
        enable_asserts=False,
        num_devices=NCORES,
    )
    w2 = nc.dram_tensor("w2", [NCHUNK, 128, SHARD], F32R, kind="ExternalInput")
    x2 = nc.dram_tensor("x2", [128, NCHUNK, TOKENS], F32R, kind="ExternalInput")
    b2 = nc.dram_tensor("b2", [TOKENS, SHARD], F32, kind="ExternalInput")
    out = nc.dram_tensor("out", [TOKENS, SHARD], F32, kind="ExternalOutput")

    with tile.TileContext(nc) as tc:
        with (
            tc.tile_pool(name="const", bufs=1) as constp,
            tc.tile_pool(name="wpool", bufs=8) as wpool,
            tc.tile_pool(name="psum", bufs=1, space="PSUM") as psump,
            tc.tile_pool(name="outp", bufs=1) as outp,
        ):
            x2sb = constp.tile([128, NCHUNK, TOKENS], F32R)
            nc.sync.dma_start(x2sb[:], x2[:])
            b2sb = constp.tile([TOKENS, SHARD], F32)
            nc.sync.dma_start(b2sb[:], b2[:])

            ptiles = [psump.tile([TOKENS, sz], F32, tag=f"ps{i}", name=f"ps{i}")
                      for i, (_, sz) in enumerate(O_SPLITS)]

            for j in range(NCHUNK):
                wj = wpool.tile([128, SHARD], F32R, tag="w")
                nc.sync.dma_start(wj[:], w2[j])
                lhs = x2sb[:, j, :]
                for pt, (off, sz) in zip(ptiles, O_SPLITS):
                    nc.tensor.matmul(
                        pt[:],
                        lhs,
                        wj[:, off:off + sz],
                        start=(j == 0),
                        stop=(j == NCHUNK - 1),
                    )

            osb = outp.tile([TOKENS, SHARD], F32)
            for pt, (off, sz) in zip(ptiles, O_SPLITS):
                nc.vector.tensor_add(osb[:, off:off + sz], pt[:], b2sb[:, off:off + sz])
            nc.sync.dma_start(out[:], osb[:])

    nc.compile()
    return nc


def _get_nc():
    if "nc" not in _CACHE:
        _CACHE["nc"] = _build_nc()
    return _CACHE["nc"]


def kernel(x, base_weight, q_delta, scales, zero_points, bias):
    global LAST_RESULTS
    x = np.asarray(x, dtype=np.float32)
    base_weight = np.asarray(base_weight, dtype=np.float32)
    q_delta = np.asarray(q_delta)
    scales = np.asarray(scales, dtype=np.float32)
    zero_points = np.asarray(zero_points, dtype=np.float32)
    bias = np.asarray(bias, dtype=np.float32)

    # host-side shard prep (fold dequant into the matmul + bias)
    S = x.sum(axis=1)                                     # [TOKENS]
    qs = q_delta.astype(np.float32) * scales[:, None]     # [OUT_F, IN_F]
    bias2 = bias[None, :] - np.outer(S, scales * zero_points)  # [TOKENS, OUT_F]

    baseT = base_weight.T                                 # [IN_F, OUT_F] view
    qsT = qs.T                                            # view

    x2T = np.concatenate([x, x], axis=1).T                # [K2, TOKENS]
    x2sb = np.ascontiguousarray(
        x2T.reshape(NCHUNK, 128, TOKENS).transpose(1, 0, 2)
    )                                                     # [128, NCHUNK, TOKENS]

    in_maps = []
    for c in range(NCORES):
        sl = slice(c * SHARD, (c + 1) * SHARD)
        w2c = np.empty((K2, SHARD), dtype=np.float32)
        w2c[:IN_F] = baseT[:, sl]
        w2c[IN_F:] = qsT[:, sl]
        in_maps.append({
            "w2": w2c.reshape(NCHUNK, 128, SHARD),
            "x2": x2sb,
            "b2": np.ascontiguousarray(bias2[:, sl]),
        })

    nc = _get_nc()
    res = bass_utils.run_bass_kernel_spmd(
        nc, in_maps, core_ids=list(range(NCORES)), trace=TRACE
    )
    LAST_RESULTS = res
    out = np.concatenate([res.results[c]["out"] for c in range(NCORES)], axis=1)
    return np.ascontiguousarray(out, dtype=np.float32)


# revision 4
# speedup vs baseline: 1.3505x; 1.3505x over previous
"""DeltaQuantLinear kernel for 8 Trainium2 NeuronCores.

Computes out = x @ (base_weight + (q_delta - zp[:,None]) * scale[:,None]).T + bias
with x [8, 4096] fp32, base_weight/q_delta [11008, 4096], per-channel
scales/zero_points/bias [11008].

Strategy (column-parallel over out_features, per the sharding hint):
  The dequant folds into pure GEMM algebra:
      out[t,o] = sum_i x[t,i]*base[o,i] + scale[o]*sum_i x[t,i]*q[o,i]
               + (bias[o] - scale[o]*zp[o]*S[t]),   S[t] = sum_i x[t,i]
  so the device kernel is a memory-bound matmul streaming the weights once
  through the PE with the tiny x operand stationary. q_delta values are
  0..15, so they ship to HBM as int8 (lossless, 4x less traffic than int32)
  and are widened to float32r on the vector engine before the PE. Per core:
  base [4096, 1376] f32 + q [4096, 1376] i8, 6 PSUM accumulators (base/q x
  3 o-splits), epilogue out = psum_b + scale*psum_q + folded_bias.
"""

import numpy as np

from concourse import bacc, bass, mybir, tile
from concourse import bass_utils

IN_F = 4096
OUT_F = 11008
TOKENS = 8
NCORES = 8
SHARD = OUT_F // NCORES          # 1376
NCHUNK = IN_F // 128             # 32 chunks of 128 along the contract dim
# o-tile split of the 1376-wide shard into <=512 psum banks
O_SPLITS = [(0, 512), (512, 512), (1024, 352)]

F32 = mybir.dt.float32
F32R = mybir.dt.float32r
I8 = mybir.dt.int8

_CACHE = {}

# test.py reads this after calling kernel() to get profile info
LAST_RESULTS = None
TRACE = False


def _build_nc():
    nc = bacc.Bacc(
        "TRN2",
        target_bir_lowering=False,
        debug=False,
        enable_asserts=False,
        num_devices=NCORES,
    )
    base = nc.dram_tensor("base", [NCHUNK, 128, SHARD], F32R, kind="ExternalInput")
    q8 = nc.dram_tensor("q8", [NCHUNK, 128, SHARD], I8, kind="ExternalInput")
    x = nc.dram_tensor("x", [128, NCHUNK, TOKENS], F32R, kind="ExternalInput")
    srep = nc.dram_tensor("srep", [TOKENS, SHARD], F32, kind="ExternalInput")
    b2 = nc.dram_tensor("b2", [TOKENS, SHARD], F32, kind="ExternalInput")
    out = nc.dram_tensor("out", [TOKENS, SHARD], F32, kind="ExternalOutput")

    with tile.TileContext(nc) as tc:
        with (
            tc.tile_pool(name="const", bufs=1) as constp,
            tc.tile_pool(name="bpool", bufs=6) as bpool,
            tc.tile_pool(name="q8pool", bufs=6) as q8pool,
            tc.tile_pool(name="qfpool", bufs=4) as qfpool,
            tc.tile_pool(name="psum", bufs=1, space="PSUM") as psump,
            tc.tile_pool(name="epi", bufs=2) as epip,
            tc.tile_pool(name="outp", bufs=1) as outp,
        ):
            xsb = constp.tile([128, NCHUNK, TOKENS], F32R)
            nc.sync.dma_start(xsb[:], x[:])
            srepsb = constp.tile([TOKENS, SHARD], F32)
            nc.sync.dma_start(srepsb[:], srep[:])
            b2sb = constp.tile([TOKENS, SHARD], F32)
            nc.sync.dma_start(b2sb[:], b2[:])

            pb = [psump.tile([TOKENS, sz], F32, tag=f"pb{i}", name=f"pb{i}")
                  for i, (_, sz) in enumerate(O_SPLITS)]
            pq = [psump.tile([TOKENS, sz], F32, tag=f"pq{i}", name=f"pq{i}")
                  for i, (_, sz) in enumerate(O_SPLITS)]

            for j in range(NCHUNK):
                bj = bpool.tile([128, SHARD], F32R, tag="b")
                nc.sync.dma_start(bj[:], base[j])
                q8j = q8pool.tile([128, SHARD], I8, tag="q8")
                nc.sync.dma_start(q8j[:], q8[j])
                qf = qfpool.tile([128, SHARD], F32R, tag="qf")
                nc.vector.tensor_copy(qf[:], q8j[:])

                lhs = xsb[:, j, :]
                first, last = j == 0, j == NCHUNK - 1
                for i, (off, sz) in enumerate(O_SPLITS):
                    nc.tensor.matmul(pb[i][:], lhs, bj[:, off:off + sz],
                                     start=first, stop=last)
                    nc.tensor.matmul(pq[i][:], lhs, qf[:, off:off + sz],
                                     start=first, stop=last)

            osb = outp.tile([TOKENS, SHARD], F32)
            for i, (off, sz) in enumerate(O_SPLITS):
                t0 = epip.tile([TOKENS, 512], F32, tag="t0", name="t0")
                nc.vector.tensor_mul(t0[:, :sz], pq[i][:], srepsb[:, off:off + sz])
                nc.vector.tensor_add(t0[:, :sz], t0[:, :sz], pb[i][:])
                nc.vector.tensor_add(osb[:, off:off + sz], t0[:, :sz],
                                     b2sb[:, off:off + sz])
            nc.sync.dma_start(out[:], osb[:])

    nc.compile()
    return nc


def _get_nc():
    if "nc" not in _CACHE:
        _CACHE["nc"] = _build_nc()
    return _CACHE["nc"]


def kernel(x, base_weight, q_delta, scales, zero_points, bias):
    global LAST_RESULTS
    x = np.asarray(x, dtype=np.float32)
    base_weight = np.asarray(base_weight, dtype=np.float32)
    q_delta = np.asarray(q_delta)
    scales = np.asarray(scales, dtype=np.float32)
    zero_points = np.asarray(zero_points, dtype=np.float32)
    bias = np.asarray(bias, dtype=np.float32)

    # host-side shard prep (fold dequant into the matmul + bias)
    S = x.sum(axis=1)                                          # [TOKENS]
    bias2 = bias[None, :] - np.outer(S, scales * zero_points)  # [TOKENS, OUT_F]
    srep_full = np.broadcast_to(scales[None, :], (TOKENS, OUT_F))

    baseT = base_weight.T                                      # [IN_F, OUT_F] view
    q8T = q_delta.astype(np.int8).T                            # values 0..15, lossless

    xsb = np.ascontiguousarray(
        x.T.reshape(NCHUNK, 128, TOKENS).transpose(1, 0, 2)
    )                                                          # [128, NCHUNK, TOKENS]

    in_maps = []
    for c in range(NCORES):
        sl = slice(c * SHARD, (c + 1) * SHARD)
        in_maps.append({
            "base": np.ascontiguousarray(baseT[:, sl]).reshape(NCHUNK, 128, SHARD),
            "q8": np.ascontiguousarray(q8T[:, sl]).reshape(NCHUNK, 128, SHARD),
            "x": xsb,
            "srep": np.ascontiguousarray(srep_full[:, sl]),
            "b2": np.ascontiguousarray(bias2[:, sl]),
        })

    nc = _get_nc()
    res = bass_utils.run_bass_kernel_spmd(
        nc, in_maps, core_ids=list(range(NCORES)), trace=TRACE
    )
    LAST_RESULTS = res
    out = np.concatenate([res.results[c]["out"] for c in range(NCORES)], axis=1)
    return np.ascontiguousarray(out, dtype=np.float32)
